# revision 11
# baseline (speedup 1.0000x reference)
"""Trainium2 Bass kernel: monomials x^a y^b z^c (a+b+c <= 3) for N=2M points.

Data-parallel across 8 NeuronCores; each core gets N/8 = 250k points padded
to 128*F*T. The trivial columns (1, x, y, z) are assembled host-side; the
device computes only the 16 degree>=2 monomials, minimizing HBM write
traffic (the binding roofline: ~358 GB/s per core).

Per tile of 128 x F points:
  in-tile  it [P, F, 3]  (point-major interleaved x,y,z; contiguous load)
  out-tile ot [P, F, 16] (point-major; contiguous store)
Device cols: 0:x2 1:xy 2:xz 3:y2 4:yz 5:z2
             6:x3 7:x2y 8:x2z 9:xy2 10:xyz 11:xz2 12:y3 13:y2z 14:yz2 15:z3
DVE (fused, step-0 broadcast in0): deg2 = x*(x,y,z)->0:3, y*(y,z)->3:5,
  z*z->5; deg3 = x*cols0:6->6:12, y*cols3:6->12:15, z*col5->15.
ACT: issues out-DMAs. SP: in-DMAs, just-in-time (front-loading all inputs
delays the output stream: the input queue has strict priority on the SDMA
engines).

Raw bass (no Tile): this walrus rejects >1 sync-wait per instruction, so all
waits are standalone wait_ge ops. Every tile has its own input slot and
sem; output slots are BO-deep with per-slot sems (one DMA in flight per sem
keeps 16*n waits unambiguous).
"""

import sys
from contextlib import ExitStack

if "/opt/trn_rl_repo" not in sys.path:
    sys.path.insert(0, "/opt/trn_rl_repo")

import numpy as np
import concourse.bass as bass
import concourse.mybir as mybir
from concourse.bass_utils import run_bass_kernel_spmd

P = 128
K = 20
KD = 16  # device-computed columns (degree >= 2)
N_TOTAL = 2_000_000
N_CORES = 8
N_CORE = N_TOTAL // N_CORES  # 250_000
F = 245
T = 8
BO = 3
N_PAD = P * F * T  # 250_880

AF = mybir.ActivationFunctionType
F32 = mybir.dt.float32
BF16 = mybir.dt.bfloat16


def build(nc: bass.Bass, n_pts: int, f: int, bo: int = BO) -> bass.Bass:
    t_total = n_pts // (P * f)
    assert t_total * P * f == n_pts

    v = nc.declare_dram_parameter("vectors", [n_pts, 3], F32, isOutput=False)
    o = nc.declare_dram_parameter("out", [n_pts, KD], BF16, isOutput=True)
    vr = v.rearrange("(t p f) c -> t p (f c)", p=P, f=f)
    orr = o.rearrange("(t p f) k -> t p (f k)", p=P, f=f)

    with ExitStack() as ctx:
        itb = ctx.enter_context(nc.sbuf_tensor("itb", [P, t_total * f * 3], F32))
        otb = ctx.enter_context(nc.sbuf_tensor("otb", [P, bo * f * KD], BF16))
        s_in = [ctx.enter_context(nc.semaphore(f"s_in{i}")) for i in range(t_total)]
        s_out = [ctx.enter_context(nc.semaphore(f"s_out{i}")) for i in range(bo)]
        s_v = ctx.enter_context(nc.semaphore("s_v"))
        s_d = ctx.enter_context(nc.semaphore("s_d"))
        s_q = ctx.enter_context(nc.semaphore("s_q"))
        block = ctx.enter_context(nc.Block(no_gpsimd_drain=True))

        def it_view(t):
            return itb.ap()[:, t * f * 3 : (t + 1) * f * 3].rearrange(
                "p (f c) -> p f c", c=3
            )

        def ot_flat(s):
            return otb.ap()[:, s * f * KD : (s + 1) * f * KD]

        def ot_view(s):
            return ot_flat(s).rearrange("p (f k) -> p f k", k=KD)

        @block.sync
        def _(sync):
            # Front-load all input DMAs: the input queue has strict priority
            # over the output queue on the SDMA engines, so interleaving
            # punches holes in the output stream. Serialized streams both
            # run at the HBM ceiling; ins finish before the first out needs
            # the engines.
            for t in range(t_total):
                sync.dma_start(
                    out=itb.ap()[:, t * f * 3 : (t + 1) * f * 3], in_=vr[t]
                ).then_inc(s_in[t], 16)

        @block.vector
        def _(vector):
            for t in range(t_total):
                s = t % bo
                n_use = t // bo  # completed uses of this out slot
                itv = it_view(t)
                otv = ot_view(s)
                x = itv[:, :, 0:1]
                y = itv[:, :, 1:2]
                z = itv[:, :, 2:3]
                vector.wait_ge(s_in[t], 16)
                if t >= bo:
                    # WAR: out-DMA of the tile previously in this slot done
                    vector.wait_ge(s_out[s], 16 * n_use)
                nc.vector.tensor_mul(
                    otv[:, :, 1:3], x.broadcast_to([P, f, 2]), itv[:, :, 1:3]
                )
                nc.vector.tensor_mul(otv[:, :, 4:5], y, z).then_inc(s_d, 1)
                # deg3 reads ACT's squares (s_q) and our own xy/xz/yz
                # through the deep DVE pipeline (s_d).
                vector.wait_ge(s_d, t + 1)
                vector.wait_ge(s_q, t + 1)
                nc.vector.tensor_mul(
                    otv[:, :, 6:12], x.broadcast_to([P, f, 6]), otv[:, :, 0:6]
                )
                nc.vector.tensor_mul(
                    otv[:, :, 12:15], y.broadcast_to([P, f, 3]), otv[:, :, 3:6]
                )
                nc.vector.tensor_mul(otv[:, :, 15:16], z, otv[:, :, 5:6]).then_inc(
                    s_v, 1
                )

        @block.scalar
        def _(scalar):
            for t in range(t_total):
                s = t % bo
                n_use = t // bo
                itv = it_view(t)
                otv = ot_view(s)
                scalar.wait_ge(s_in[t], 16)
                if t >= bo:
                    scalar.wait_ge(s_out[s], 16 * n_use)
                nc.scalar.square(otv[:, :, 0:1], itv[:, :, 0:1])
                nc.scalar.square(otv[:, :, 3:4], itv[:, :, 1:2])
                nc.scalar.square(otv[:, :, 5:6], itv[:, :, 2:3]).then_inc(s_q, 1)
                scalar.wait_ge(s_v, t + 1)
                scalar.dma_start(out=orr[t], in_=ot_flat(s)).then_inc(s_out[s], 16)
            for s in range(bo):
                uses = len([t for t in range(t_total) if t % bo == s])
                if uses:
                    scalar.wait_ge(s_out[s], 16 * uses)

    return nc


_CACHE: dict[str, object] = {}


def _get_nc() -> bass.Bass:
    if "nc" not in _CACHE:
        nc = bass.Bass()
        build(nc, N_PAD, F, BO)
        _CACHE["nc"] = nc
    return _CACHE["nc"]  # type: ignore[return-value]


def run_spmd(in_maps, trace=False, **kw):
    return run_bass_kernel_spmd(
        _get_nc(), in_maps, core_ids=list(range(N_CORES)), trace=trace, **kw
    )


def make_in_maps(vectors: np.ndarray):
    vectors = np.ascontiguousarray(np.asarray(vectors, dtype=np.float32))
    assert vectors.shape == (N_TOTAL, 3)
    shards = vectors.reshape(N_CORES, N_CORE, 3)
    in_maps = []
    for i in range(N_CORES):
        buf = np.zeros((N_PAD, 3), dtype=np.float32)
        buf[:N_CORE] = shards[i]
        in_maps.append({"vectors": buf})
    return in_maps


def kernel(vectors: np.ndarray) -> np.ndarray:
    vec32 = np.ascontiguousarray(np.asarray(vectors, dtype=np.float32))
    res = run_spmd(make_in_maps(vec32))
    out = np.empty((N_TOTAL, K), dtype=np.float32)
    out[:, 0] = 1.0
    out[:, 1:4] = vec32  # degree-1 monomials are the input, exactly
    for i in range(N_CORES):
        out[i * N_CORE : (i + 1) * N_CORE, 4:] = np.asarray(
            res.results[i]["out"][:N_CORE], dtype=np.float32
        )
    return out


# revision 12
# speedup vs baseline: 1.1962x; 1.1962x over previous
"""Trainium2 Bass kernel: monomials x^a y^b z^c (a+b+c <= 3) for N=2M points.

Data-parallel across 8 NeuronCores; each core gets N/8 = 250k points padded
to 128*F*T. The trivial columns (1, x, y, z) are assembled host-side; the
device computes only the 16 degree>=2 monomials, minimizing HBM write
traffic (the binding roofline: ~358 GB/s per core).

Per tile of 128 x F points:
  in-tile  it [P, F, 3]  (point-major interleaved x,y,z; contiguous load)
  out-tile ot [P, F, 16] (point-major; contiguous store)
Device cols: 0:x2 1:xy 2:xz 3:y2 4:yz 5:z2
             6:x3 7:x2y 8:x2z 9:xy2 10:xyz 11:xz2 12:y3 13:y2z 14:yz2 15:z3
DVE (fused, step-0 broadcast in0): deg2 = x*(x,y,z)->0:3, y*(y,z)->3:5,
  z*z->5; deg3 = x*cols0:6->6:12, y*cols3:6->12:15, z*col5->15.
ACT: issues out-DMAs. SP: in-DMAs, just-in-time (front-loading all inputs
delays the output stream: the input queue has strict priority on the SDMA
engines).

Raw bass (no Tile): this walrus rejects >1 sync-wait per instruction, so all
waits are standalone wait_ge ops. Every tile has its own input slot and
sem; output slots are BO-deep with per-slot sems (one DMA in flight per sem
keeps 16*n waits unambiguous).
"""

import sys
from contextlib import ExitStack

if "/opt/trn_rl_repo" not in sys.path:
    sys.path.insert(0, "/opt/trn_rl_repo")

import numpy as np
import concourse.bass as bass
import concourse.mybir as mybir
from concourse.bass_utils import run_bass_kernel_spmd

P = 128
K = 20
KD = 16  # device-computed columns (degree >= 2)
N_TOTAL = 2_000_000
N_CORES = 8
N_CORE = N_TOTAL // N_CORES  # 250_000
F = 245
T = 8
BO = 3
N_PAD = P * F * T  # 250_880

AF = mybir.ActivationFunctionType
F32 = mybir.dt.float32
BF16 = mybir.dt.bfloat16


def build(nc: bass.Bass, n_pts: int, f: int, bo: int = BO) -> bass.Bass:
    t_total = n_pts // (P * f)
    assert t_total * P * f == n_pts

    v = nc.declare_dram_parameter("vectors", [n_pts, 3], F32, isOutput=False)
    o = nc.declare_dram_parameter("out", [n_pts, KD], BF16, isOutput=True)
    vr = v.rearrange("(t p f) c -> t p (f c)", p=P, f=f)
    orr = o.rearrange("(t p f) k -> t p (f k)", p=P, f=f)

    with ExitStack() as ctx:
        itb = ctx.enter_context(nc.sbuf_tensor("itb", [P, t_total * f * 3], F32))
        otb = ctx.enter_context(nc.sbuf_tensor("otb", [P, bo * f * KD], BF16))
        s_in = [ctx.enter_context(nc.semaphore(f"s_in{i}")) for i in range(t_total)]
        s_out = [ctx.enter_context(nc.semaphore(f"s_out{i}")) for i in range(bo)]
        s_v = ctx.enter_context(nc.semaphore("s_v"))
        s_d = ctx.enter_context(nc.semaphore("s_d"))
        s_q = ctx.enter_context(nc.semaphore("s_q"))
        block = ctx.enter_context(nc.Block(no_gpsimd_drain=True))

        def it_view(t):
            return itb.ap()[:, t * f * 3 : (t + 1) * f * 3].rearrange(
                "p (f c) -> p f c", c=3
            )

        def ot_flat(s):
            return otb.ap()[:, s * f * KD : (s + 1) * f * KD]

        def ot_view(s):
            return ot_flat(s).rearrange("p (f k) -> p f k", k=KD)

        @block.sync
        def _(sync):
            # Front-load all input DMAs (ins finish before outs need the
            # SDMA engines), then trigger out-DMAs from here: SP is
            # otherwise idle, so DVE/ACT never block on a DMA trigger
            # waiting for the other engine.
            for t in range(t_total):
                sync.dma_start(
                    out=itb.ap()[:, t * f * 3 : (t + 1) * f * 3], in_=vr[t]
                ).then_inc(s_in[t], 16)
            for t in range(t_total):
                s = t % bo
                sync.wait_ge(s_v, t + 1)
                sync.dma_start(out=orr[t], in_=ot_flat(s)).then_inc(s_out[s], 16)
            for s in range(bo):
                uses = len([t for t in range(t_total) if t % bo == s])
                if uses:
                    sync.wait_ge(s_out[s], 16 * uses)

        @block.vector
        def _(vector):
            for t in range(t_total):
                s = t % bo
                n_use = t // bo  # completed uses of this out slot
                itv = it_view(t)
                otv = ot_view(s)
                x = itv[:, :, 0:1]
                y = itv[:, :, 1:2]
                z = itv[:, :, 2:3]
                vector.wait_ge(s_in[t], 16)
                if t >= bo:
                    # WAR: out-DMA of the tile previously in this slot done
                    vector.wait_ge(s_out[s], 16 * n_use)
                nc.vector.tensor_mul(
                    otv[:, :, 1:3], x.broadcast_to([P, f, 2]), itv[:, :, 1:3]
                )
                nc.vector.tensor_mul(otv[:, :, 4:5], y, z).then_inc(s_d, 1)
                # deg3 reads ACT's squares (s_q) and our own xy/xz/yz
                # through the deep DVE pipeline (s_d).
                vector.wait_ge(s_d, t + 1)
                vector.wait_ge(s_q, t + 1)
                nc.vector.tensor_mul(
                    otv[:, :, 6:12], x.broadcast_to([P, f, 6]), otv[:, :, 0:6]
                )
                nc.vector.tensor_mul(
                    otv[:, :, 12:15], y.broadcast_to([P, f, 3]), otv[:, :, 3:6]
                )
                nc.vector.tensor_mul(otv[:, :, 15:16], z, otv[:, :, 5:6]).then_inc(
                    s_v, 1
                )

        @block.scalar
        def _(scalar):
            for t in range(t_total):
                s = t % bo
                n_use = t // bo
                itv = it_view(t)
                otv = ot_view(s)
                scalar.wait_ge(s_in[t], 16)
                if t >= bo:
                    scalar.wait_ge(s_out[s], 16 * n_use)
                nc.scalar.square(otv[:, :, 0:1], itv[:, :, 0:1])
                nc.scalar.square(otv[:, :, 3:4], itv[:, :, 1:2])
                nc.scalar.square(otv[:, :, 5:6], itv[:, :, 2:3]).then_inc(s_q, 1)

    return nc


_CACHE: dict[str, object] = {}


def _get_nc() -> bass.Bass:
    if "nc" not in _CACHE:
        nc = bass.Bass()
        build(nc, N_PAD, F, BO)
        _CACHE["nc"] = nc
    return _CACHE["nc"]  # type: ignore[return-value]


def run_spmd(in_maps, trace=False, **kw):
    return run_bass_kernel_spmd(
        _get_nc(), in_maps, core_ids=list(range(N_CORES)), trace=trace, **kw
    )


def make_in_maps(vectors: np.ndarray):
    vectors = np.ascontiguousarray(np.asarray(vectors, dtype=np.float32))
    assert vectors.shape == (N_TOTAL, 3)
    shards = vectors.reshape(N_CORES, N_CORE, 3)
    in_maps = []
    for i in range(N_CORES):
        buf = np.zeros((N_PAD, 3), dtype=np.float32)
        buf[:N_CORE] = shards[i]
        in_maps.append({"vectors": buf})
    return in_maps


def kernel(vectors: np.ndarray) -> np.ndarray:
    vec32 = np.ascontiguousarray(np.asarray(vectors, dtype=np.float32))
    res = run_spmd(make_in_maps(vec32))
    out = np.empty((N_TOTAL, K), dtype=np.float32)
    out[:, 0] = 1.0
    out[:, 1:4] = vec32  # degree-1 monomials are the input, exactly
    for i in range(N_CORES):
        out[i * N_CORE : (i + 1) * N_CORE, 4:] = np.asarray(
            res.results[i]["out"][:N_CORE], dtype=np.float32
        )
    return out


# revision 13
# speedup vs baseline: 1.5659x; 1.3091x over previous
"""Trainium2 Bass kernel: monomials x^a y^b z^c (a+b+c <= 3) for N=2M points.

Data-parallel across 8 NeuronCores; each core gets N/8 = 250k points padded
to 128*F*T. The trivial columns (1, x, y, z) are assembled host-side; the
device computes the 16 degree>=2 monomials in bf16 (one rounding vs the
f32 reference, ~8e-3 max rel err, well under the 2e-2 gate) to halve the
HBM write traffic.

Layout is PLANAR to keep every engine access unit-stride with long runs
(strided APs with short inner counts run at 2-5 cyc/elem on DVE/ACT):
  host in  : per tile [128, 3, F]  (x-plane, y-plane, z-plane per partition)
  SBUF it  : [P, 3F] f32           x = [0:F], y = [F:2F], z = [2F:3F]
  SBUF ot  : [P, 16F] bf16         monomial k = [kF:(k+1)F]
  host out : per tile [128, 16, F] -> transposed to [points, 16] on host
Device monomials: 0:x2 1:xy 2:xz 3:y2 4:yz 5:z2
             6:x3 7:x2y 8:x2z 9:xy2 10:xyz 11:xz2 12:y3 13:y2z 14:yz2 15:z3
ACT: squares -> planes 0,3,5. DVE: products (in0 broadcast along a step-0
middle dim): xy|xz <- x*(y,z); yz; deg3 = x*(planes0:6)->6:12,
y*(planes3:6)->12:15, z*plane5->15. SP: all DMAs + out triggers (keeps
DVE/ACT from blocking on each other's DMA waits).

Raw bass (no Tile): this walrus rejects >1 sync-wait per instruction, so
all waits are standalone wait_ge ops. Every tile has its own input slot and
sem; output slots are BO-deep with per-slot sems (one DMA in flight per sem
keeps 16*n waits unambiguous).
"""

import sys
from contextlib import ExitStack

if "/opt/trn_rl_repo" not in sys.path:
    sys.path.insert(0, "/opt/trn_rl_repo")

import numpy as np
import concourse.bass as bass
import concourse.mybir as mybir
from concourse.bass_utils import run_bass_kernel_spmd

P = 128
K = 20
KD = 16  # device-computed columns (degree >= 2)
N_TOTAL = 2_000_000
N_CORES = 8
N_CORE = N_TOTAL // N_CORES  # 250_000
F = 490
T = 4
BO = 3
N_PAD = P * F * T  # 250_880

AF = mybir.ActivationFunctionType
F32 = mybir.dt.float32
BF16 = mybir.dt.bfloat16


def build(nc: bass.Bass, n_pts: int, f: int, bo: int = BO) -> bass.Bass:
    t_total = n_pts // (P * f)
    assert t_total * P * f == n_pts

    v = nc.declare_dram_parameter("vectors", [t_total * P, 3 * f], F32, isOutput=False)
    o = nc.declare_dram_parameter("out", [t_total * P, KD * f], BF16, isOutput=True)

    with ExitStack() as ctx:
        itb = ctx.enter_context(nc.sbuf_tensor("itb", [P, t_total * 3 * f], F32))
        otb = ctx.enter_context(nc.sbuf_tensor("otb", [P, bo * KD * f], BF16))
        s_in = [ctx.enter_context(nc.semaphore(f"s_in{i}")) for i in range(t_total)]
        s_out = [ctx.enter_context(nc.semaphore(f"s_out{i}")) for i in range(bo)]
        s_v = ctx.enter_context(nc.semaphore("s_v"))
        s_d = ctx.enter_context(nc.semaphore("s_d"))
        s_q = ctx.enter_context(nc.semaphore("s_q"))
        block = ctx.enter_context(nc.Block(no_gpsimd_drain=True))

        def it_flat(t):
            return itb.ap()[:, t * 3 * f : (t + 1) * 3 * f]

        def ot_flat(s):
            return otb.ap()[:, s * KD * f : (s + 1) * KD * f]

        def plane(base, k, w=1):
            """[P, w, f] view of planes k..k+w of a flat [P, n*f] AP."""
            return base[:, k * f : (k + w) * f].rearrange("p (c f) -> p c f", f=f)

        def bcast(base, k, w):
            """plane k broadcast w times along a step-0 middle dim."""
            return plane(base, k, 1).broadcast_to([P, w, f])

        @block.sync
        def _(sync):
            # Front-load all input DMAs (ins finish before outs need the
            # SDMA engines), then trigger out-DMAs from here: SP is
            # otherwise idle, so DVE/ACT never block on a DMA trigger
            # waiting for the other engine.
            for t in range(t_total):
                sync.dma_start(
                    out=it_flat(t), in_=v[t * P : (t + 1) * P, :]
                ).then_inc(s_in[t], 16)
            for t in range(t_total):
                s = t % bo
                sync.wait_ge(s_v, t + 1)
                sync.dma_start(
                    out=o[t * P : (t + 1) * P, :], in_=ot_flat(s)
                ).then_inc(s_out[s], 16)
            for s in range(bo):
                uses = len([t for t in range(t_total) if t % bo == s])
                if uses:
                    sync.wait_ge(s_out[s], 16 * uses)

        @block.vector
        def _(vector):
            for t in range(t_total):
                s = t % bo
                n_use = t // bo  # completed uses of this out slot
                it = it_flat(t)
                ot = ot_flat(s)
                vector.wait_ge(s_in[t], 16)
                if t >= bo:
                    # WAR: out-DMA of the tile previously in this slot done
                    vector.wait_ge(s_out[s], 16 * n_use)
                # xy, xz -> planes 1:3
                nc.vector.tensor_mul(plane(ot, 1, 2), bcast(it, 0, 2), plane(it, 1, 2))
                # yz -> plane 4
                nc.vector.tensor_mul(
                    plane(ot, 4), plane(it, 1), plane(it, 2)
                ).then_inc(s_d, 1)
                # deg3 reads ACT's squares (s_q) and our own xy/xz/yz
                # through the deep DVE pipeline (s_d).
                vector.wait_ge(s_d, t + 1)
                vector.wait_ge(s_q, t + 1)
                nc.vector.tensor_mul(plane(ot, 6, 6), bcast(it, 0, 6), plane(ot, 0, 6))
                nc.vector.tensor_mul(plane(ot, 12, 3), bcast(it, 1, 3), plane(ot, 3, 3))
                nc.vector.tensor_mul(
                    plane(ot, 15), plane(it, 2), plane(ot, 5)
                ).then_inc(s_v, 1)

        @block.scalar
        def _(scalar):
            for t in range(t_total):
                s = t % bo
                n_use = t // bo
                it = it_flat(t)
                ot = ot_flat(s)
                scalar.wait_ge(s_in[t], 16)
                if t >= bo:
                    scalar.wait_ge(s_out[s], 16 * n_use)
                nc.scalar.square(plane(ot, 0), plane(it, 0))
                nc.scalar.square(plane(ot, 3), plane(it, 1))
                nc.scalar.square(plane(ot, 5), plane(it, 2)).then_inc(s_q, 1)

    return nc


_CACHE: dict[str, object] = {}


def _get_nc() -> bass.Bass:
    if "nc" not in _CACHE:
        nc = bass.Bass()
        build(nc, N_PAD, F, BO)
        _CACHE["nc"] = nc
    return _CACHE["nc"]  # type: ignore[return-value]


def run_spmd(in_maps, trace=False, **kw):
    return run_bass_kernel_spmd(
        _get_nc(), in_maps, core_ids=list(range(N_CORES)), trace=trace, **kw
    )


def to_planar(shard: np.ndarray, f: int = F, t_total: int = T) -> np.ndarray:
    """[n_pad, 3] f32 -> [t*P, 3*f] planar (x,y,z planes per partition)."""
    a = shard.reshape(t_total, P, f, 3).transpose(0, 1, 3, 2)  # [T,P,3,F]
    return np.ascontiguousarray(a.reshape(t_total * P, 3 * f))


def from_planar(dev_out: np.ndarray, f: int = F, t_total: int = T) -> np.ndarray:
    """[t*P, 16*f] (any dtype) -> [n_pad, 16] f32."""
    a = np.asarray(dev_out, dtype=np.float32).reshape(t_total, P, KD, f)
    return a.transpose(0, 1, 3, 2).reshape(t_total * P * f, KD)


def make_in_maps(vectors: np.ndarray):
    vectors = np.ascontiguousarray(np.asarray(vectors, dtype=np.float32))
    assert vectors.shape == (N_TOTAL, 3)
    shards = vectors.reshape(N_CORES, N_CORE, 3)
    in_maps = []
    for i in range(N_CORES):
        buf = np.zeros((N_PAD, 3), dtype=np.float32)
        buf[:N_CORE] = shards[i]
        in_maps.append({"vectors": to_planar(buf)})
    return in_maps


def kernel(vectors: np.ndarray) -> np.ndarray:
    vec32 = np.ascontiguousarray(np.asarray(vectors, dtype=np.float32))
    res = run_spmd(make_in_maps(vec32))
    out = np.empty((N_TOTAL, K), dtype=np.float32)
    out[:, 0] = 1.0
    out[:, 1:4] = vec32  # degree-1 monomials are the input, exactly
    for i in range(N_CORES):
        out[i * N_CORE : (i + 1) * N_CORE, 4:] = from_planar(res.results[i]["out"])[
            :N_CORE
        ]
    return out


# revision 14
# speedup vs baseline: 1.7057x; 1.0893x over previous
"""Trainium2 Bass kernel: monomials x^a y^b z^c (a+b+c <= 3) for N=2M points.

Data-parallel across 8 NeuronCores; each core gets N/8 = 250k points padded
to 128*F*T. The trivial columns (1, x, y, z) are assembled host-side; the
device computes the 16 degree>=2 monomials in bf16 (one rounding vs the
f32 reference, ~8e-3 max rel err, well under the 2e-2 gate) to halve the
HBM write traffic.

Layout is PLANAR to keep every engine access unit-stride with long runs
(strided APs with short inner counts run at 2-5 cyc/elem on DVE/ACT):
  host in  : per tile [128, 3, F]  (x-plane, y-plane, z-plane per partition)
  SBUF it  : [P, 3F] f32           x = [0:F], y = [F:2F], z = [2F:3F]
  SBUF ot  : [P, 16F] bf16         monomial k = [kF:(k+1)F]
  host out : per tile [128, 16, F] -> transposed to [points, 16] on host
Device monomials: 0:x2 1:xy 2:xz 3:y2 4:yz 5:z2
             6:x3 7:x2y 8:x2z 9:xy2 10:xyz 11:xz2 12:y3 13:y2z 14:yz2 15:z3
ACT: squares -> planes 0,3,5. DVE: products (in0 broadcast along a step-0
middle dim): xy|xz <- x*(y,z); yz; deg3 = x*(planes0:6)->6:12,
y*(planes3:6)->12:15, z*plane5->15. SP: all DMAs + out triggers (keeps
DVE/ACT from blocking on each other's DMA waits).

Raw bass (no Tile): this walrus rejects >1 sync-wait per instruction, so
all waits are standalone wait_ge ops. Every tile has its own input slot and
sem; output slots are BO-deep with per-slot sems (one DMA in flight per sem
keeps 16*n waits unambiguous).
"""

import sys
from contextlib import ExitStack

if "/opt/trn_rl_repo" not in sys.path:
    sys.path.insert(0, "/opt/trn_rl_repo")

import numpy as np
import concourse.bass as bass
import concourse.mybir as mybir
from concourse.bass_utils import run_bass_kernel_spmd

P = 128
K = 20
KD = 16  # device-computed columns (degree >= 2)
N_TOTAL = 2_000_000
N_CORES = 8
N_CORE = N_TOTAL // N_CORES  # 250_000
F = 490
T = 4
BO = 4  # == T: every tile has its own out slot, no WAR waits
N_PAD = P * F * T  # 250_880

AF = mybir.ActivationFunctionType
F32 = mybir.dt.float32
BF16 = mybir.dt.bfloat16


def build(nc: bass.Bass, n_pts: int, f: int, bo: int = BO) -> bass.Bass:
    t_total = n_pts // (P * f)
    assert t_total * P * f == n_pts

    v = nc.declare_dram_parameter("vectors", [t_total * P, 3 * f], F32, isOutput=False)
    o = nc.declare_dram_parameter("out", [t_total * P, KD * f], BF16, isOutput=True)

    with ExitStack() as ctx:
        itb = ctx.enter_context(nc.sbuf_tensor("itb", [P, t_total * 3 * f], F32))
        otb = ctx.enter_context(nc.sbuf_tensor("otb", [P, bo * KD * f], BF16))
        s_in = [ctx.enter_context(nc.semaphore(f"s_in{i}")) for i in range(t_total)]
        s_out = [ctx.enter_context(nc.semaphore(f"s_out{i}")) for i in range(bo)]
        s_v = ctx.enter_context(nc.semaphore("s_v"))
        s_va = ctx.enter_context(nc.semaphore("s_va"))
        s_d = ctx.enter_context(nc.semaphore("s_d"))
        s_q = ctx.enter_context(nc.semaphore("s_q"))
        block = ctx.enter_context(nc.Block(no_gpsimd_drain=True))

        def it_flat(t):
            return itb.ap()[:, t * 3 * f : (t + 1) * 3 * f]

        def ot_flat(s):
            return otb.ap()[:, s * KD * f : (s + 1) * KD * f]

        def plane(base, k, w=1):
            """[P, w, f] view of planes k..k+w of a flat [P, n*f] AP."""
            return base[:, k * f : (k + w) * f].rearrange("p (c f) -> p c f", f=f)

        def bcast(base, k, w):
            """plane k broadcast w times along a step-0 middle dim."""
            return plane(base, k, 1).broadcast_to([P, w, f])

        @block.sync
        def _(sync):
            # Front-load all input DMAs (ins finish before outs need the
            # SDMA engines), then trigger out-DMAs from here: SP is
            # otherwise idle, so DVE/ACT never block on a DMA trigger
            # waiting for the other engine.
            for t in range(t_total):
                sync.dma_start(
                    out=it_flat(t), in_=v[t * P : (t + 1) * P, :]
                ).then_inc(s_in[t], 16)
            for t in range(t_total):
                s = t % bo
                # Planes 0:12 are done after the 6-wide deg3 op (s_va);
                # 12:16 after the tile completes (s_v). Splitting starts
                # the store earlier and shortens the tail.
                sync.wait_ge(s_va, t + 1)
                sync.dma_start(
                    out=o[t * P : (t + 1) * P, 0 : 12 * f],
                    in_=ot_flat(s)[:, 0 : 12 * f],
                ).then_inc(s_out[s], 16)
                sync.wait_ge(s_v, t + 1)
                sync.dma_start(
                    out=o[t * P : (t + 1) * P, 12 * f : KD * f],
                    in_=ot_flat(s)[:, 12 * f : KD * f],
                ).then_inc(s_out[s], 16)
            for s in range(bo):
                uses = len([t for t in range(t_total) if t % bo == s])
                if uses:
                    sync.wait_ge(s_out[s], 32 * uses)

        @block.vector
        def _(vector):
            for t in range(t_total):
                s = t % bo
                n_use = t // bo  # completed uses of this out slot
                it = it_flat(t)
                ot = ot_flat(s)
                vector.wait_ge(s_in[t], 16)
                if t >= bo:
                    # WAR: both half-DMAs of the previous tile in this slot
                    vector.wait_ge(s_out[s], 32 * n_use)
                # xy, xz -> planes 1:3
                nc.vector.tensor_mul(plane(ot, 1, 2), bcast(it, 0, 2), plane(it, 1, 2))
                # yz -> plane 4
                nc.vector.tensor_mul(
                    plane(ot, 4), plane(it, 1), plane(it, 2)
                ).then_inc(s_d, 1)
                # deg3 reads ACT's squares (s_q) and our own xy/xz/yz
                # through the deep DVE pipeline (s_d).
                vector.wait_ge(s_d, t + 1)
                vector.wait_ge(s_q, t + 1)
                nc.vector.tensor_mul(
                    plane(ot, 6, 6), bcast(it, 0, 6), plane(ot, 0, 6)
                ).then_inc(s_va, 1)
                nc.vector.tensor_mul(plane(ot, 12, 3), bcast(it, 1, 3), plane(ot, 3, 3))
                nc.vector.tensor_mul(
                    plane(ot, 15), plane(it, 2), plane(ot, 5)
                ).then_inc(s_v, 1)

        @block.scalar
        def _(scalar):
            for t in range(t_total):
                s = t % bo
                n_use = t // bo
                it = it_flat(t)
                ot = ot_flat(s)
                scalar.wait_ge(s_in[t], 16)
                if t >= bo:
                    scalar.wait_ge(s_out[s], 32 * n_use)
                nc.scalar.square(plane(ot, 0), plane(it, 0))
                nc.scalar.square(plane(ot, 3), plane(it, 1))
                nc.scalar.square(plane(ot, 5), plane(it, 2)).then_inc(s_q, 1)

    return nc


_CACHE: dict[str, object] = {}


def _get_nc() -> bass.Bass:
    if "nc" not in _CACHE:
        nc = bass.Bass()
        build(nc, N_PAD, F, BO)
        _CACHE["nc"] = nc
    return _CACHE["nc"]  # type: ignore[return-value]


def run_spmd(in_maps, trace=False, **kw):
    return run_bass_kernel_spmd(
        _get_nc(), in_maps, core_ids=list(range(N_CORES)), trace=trace, **kw
    )


def to_planar(shard: np.ndarray, f: int = F, t_total: int = T) -> np.ndarray:
    """[n_pad, 3] f32 -> [t*P, 3*f] planar (x,y,z planes per partition)."""
    a = shard.reshape(t_total, P, f, 3).transpose(0, 1, 3, 2)  # [T,P,3,F]
    return np.ascontiguousarray(a.reshape(t_total * P, 3 * f))


def from_planar(dev_out: np.ndarray, f: int = F, t_total: int = T) -> np.ndarray:
    """[t*P, 16*f] (any dtype) -> [n_pad, 16] f32."""
    a = np.asarray(dev_out, dtype=np.float32).reshape(t_total, P, KD, f)
    return a.transpose(0, 1, 3, 2).reshape(t_total * P * f, KD)


def make_in_maps(vectors: np.ndarray):
    vectors = np.ascontiguousarray(np.asarray(vectors, dtype=np.float32))
    assert vectors.shape == (N_TOTAL, 3)
    shards = vectors.reshape(N_CORES, N_CORE, 3)
    in_maps = []
    for i in range(N_CORES):
        buf = np.zeros((N_PAD, 3), dtype=np.float32)
        buf[:N_CORE] = shards[i]
        in_maps.append({"vectors": to_planar(buf)})
    return in_maps


def kernel(vectors: np.ndarray) -> np.ndarray:
    vec32 = np.ascontiguousarray(np.asarray(vectors, dtype=np.float32))
    res = run_spmd(make_in_maps(vec32))
    out = np.empty((N_TOTAL, K), dtype=np.float32)
    out[:, 0] = 1.0
    out[:, 1:4] = vec32  # degree-1 monomials are the input, exactly
    for i in range(N_CORES):
        out[i * N_CORE : (i + 1) * N_CORE, 4:] = from_planar(res.results[i]["out"])[
            :N_CORE
        ]
    return out


# revision 16
# speedup vs baseline: 1.7580x; 1.0306x over previous
"""Trainium2 Bass kernel: monomials x^a y^b z^c (a+b+c <= 3) for N=2M points.

Data-parallel across 8 NeuronCores; each core gets N/8 = 250k points padded
to 128*1960. The trivial columns (1, x, y, z) are assembled host-side; the
device computes the 16 degree>=2 monomials in bf16 (one truncation vs the
f32 reference, ~8e-3 max rel err, well under the 2e-2 gate) to halve the
HBM write traffic.

Layout is PLANAR to keep every engine access unit-stride with long runs
(strided APs with short inner counts run at 2-5 cyc/elem on DVE/ACT):
  host in  : per tile [128, 3, f]  (x-plane, y-plane, z-plane per partition)
  SBUF it  : [P, 3f] f32           x = [0:f], y = [f:2f], z = [2f:3f]
  SBUF ot  : [P, 16f] bf16         monomial k = [kf:(k+1)f]
  host out : per tile [128, 16, f] -> transposed to [points, 16] on host
Device monomials: 0:x2 1:xy 2:xz 3:y2 4:yz 5:z2
             6:x3 7:x2y 8:x2z 9:xy2 10:xyz 11:xz2 12:y3 13:y2z 14:yz2 15:z3
ACT: squares -> planes 0,3,5. DVE: products (in0 broadcast along a step-0
middle dim): xy|xz <- x*(y,z); yz; deg3 = x*(planes0:6)->6:12,
y*(planes3:6)->12:15, z*plane5->15. SP: all DMAs + out triggers (keeps
DVE/ACT from blocking on each other's DMA waits).

Tiles have VARIABLE sizes (F_LIST): a small first tile fills the pipeline
fast (ramp) and a small last tile shortens the store tail. All tiles are
SBUF-resident (no slot reuse -> no WAR waits). Out-DMAs are split into
planes 0:12 (ready after the 6-wide deg3 op) and 12:16.

Raw bass (no Tile): this walrus rejects >1 sync-wait per instruction, so
all waits are standalone wait_ge ops; DMA sems are per-tile so at most one
DMA in flight per sem keeps wait values unambiguous.
"""

import sys
from contextlib import ExitStack

if "/opt/trn_rl_repo" not in sys.path:
    sys.path.insert(0, "/opt/trn_rl_repo")

import numpy as np
import concourse.bass as bass
import concourse.mybir as mybir
from concourse.bass_utils import run_bass_kernel_spmd

P = 128
K = 20
KD = 16  # device-computed columns (degree >= 2)
N_TOTAL = 2_000_000
N_CORES = 8
N_CORE = N_TOTAL // N_CORES  # 250_000
F_TOTAL = 1960
F_LIST = [98, 588, 637, 539, 98]  # sums to F_TOTAL
N_PAD = P * F_TOTAL  # 250_880

AF = mybir.ActivationFunctionType
F32 = mybir.dt.float32
BF16 = mybir.dt.bfloat16


def build(nc: bass.Bass, f_list) -> bass.Bass:
    t_total = len(f_list)
    f_sum = sum(f_list)
    offs = np.concatenate([[0], np.cumsum(f_list)]).astype(int)  # per-partition

    v = nc.declare_dram_parameter("vectors", [P * 3 * f_sum], F32, isOutput=False)
    o = nc.declare_dram_parameter("out", [P * KD * f_sum], BF16, isOutput=True)

    with ExitStack() as ctx:
        itb = ctx.enter_context(nc.sbuf_tensor("itb", [P, 3 * f_sum], F32))
        otb = ctx.enter_context(nc.sbuf_tensor("otb", [P, KD * f_sum], BF16))
        s_in = [ctx.enter_context(nc.semaphore(f"s_in{i}")) for i in range(t_total)]
        s_out = [ctx.enter_context(nc.semaphore(f"s_out{i}")) for i in range(t_total)]
        s_v = ctx.enter_context(nc.semaphore("s_v"))
        s_va = ctx.enter_context(nc.semaphore("s_va"))
        s_d = ctx.enter_context(nc.semaphore("s_d"))
        s_q = ctx.enter_context(nc.semaphore("s_q"))
        block = ctx.enter_context(nc.Block(no_gpsimd_drain=True))

        def it_flat(t):
            return itb.ap()[:, 3 * offs[t] : 3 * offs[t + 1]]

        def ot_flat(t):
            return otb.ap()[:, KD * offs[t] : KD * offs[t + 1]]

        def v_dram(t):
            return v[P * 3 * offs[t] : P * 3 * offs[t + 1]].rearrange(
                "(p q) -> p q", p=P
            )

        def plane(base, t, k, w=1):
            """[P, w, f_t] view of planes k..k+w of a per-tile flat AP."""
            f = f_list[t]
            return base[:, k * f : (k + w) * f].rearrange("p (c f) -> p c f", f=f)

        def bcast(base, t, k, w):
            f = f_list[t]
            return plane(base, t, k, 1).broadcast_to([P, w, f])

        @block.sync
        def _(sync):
            # Front-load all input DMAs (ins finish before outs need the
            # SDMA engines), then trigger out-DMAs from here: SP is
            # otherwise idle, so DVE/ACT never block on a DMA trigger
            # waiting for the other engine.
            for t in range(t_total):
                sync.dma_start(out=it_flat(t), in_=v_dram(t)).then_inc(s_in[t], 16)
            for t in range(t_total):
                f = f_list[t]
                base = P * KD * offs[t]
                od = o[base : base + P * KD * f].rearrange("(p q) -> p q", p=P)
                # Planes 0:12 are done after the 6-wide deg3 op (s_va);
                # 12:16 after the tile completes (s_v). Splitting starts
                # the store earlier and shortens the tail.
                sync.wait_ge(s_va, t + 1)
                sync.dma_start(
                    out=od[:, 0 : 12 * f], in_=ot_flat(t)[:, 0 : 12 * f]
                ).then_inc(s_out[t], 16)
                sync.wait_ge(s_v, t + 1)
                sync.dma_start(
                    out=od[:, 12 * f : KD * f], in_=ot_flat(t)[:, 12 * f : KD * f]
                ).then_inc(s_out[t], 16)
            for t in range(t_total):
                sync.wait_ge(s_out[t], 32)

        @block.vector
        def _(vector):
            for t in range(t_total):
                it = it_flat(t)
                ot = ot_flat(t)
                vector.wait_ge(s_in[t], 16)
                # xy, xz -> planes 1:3
                nc.vector.tensor_mul(
                    plane(ot, t, 1, 2), bcast(it, t, 0, 2), plane(it, t, 1, 2)
                )
                # yz -> plane 4
                nc.vector.tensor_mul(
                    plane(ot, t, 4), plane(it, t, 1), plane(it, t, 2)
                ).then_inc(s_d, 1)
                # deg3 reads ACT's squares (s_q) and our own xy/xz/yz
                # through the deep DVE pipeline (s_d).
                vector.wait_ge(s_d, t + 1)
                vector.wait_ge(s_q, t + 1)
                nc.vector.tensor_mul(
                    plane(ot, t, 6, 6), bcast(it, t, 0, 6), plane(ot, t, 0, 6)
                ).then_inc(s_va, 1)
                nc.vector.tensor_mul(
                    plane(ot, t, 12, 3), bcast(it, t, 1, 3), plane(ot, t, 3, 3)
                )
                nc.vector.tensor_mul(
                    plane(ot, t, 15), plane(it, t, 2), plane(ot, t, 5)
                ).then_inc(s_v, 1)

        @block.scalar
        def _(scalar):
            for t in range(t_total):
                it = it_flat(t)
                ot = ot_flat(t)
                scalar.wait_ge(s_in[t], 16)
                nc.scalar.square(plane(ot, t, 0), plane(it, t, 0))
                nc.scalar.square(plane(ot, t, 3), plane(it, t, 1))
                nc.scalar.square(plane(ot, t, 5), plane(it, t, 2)).then_inc(s_q, 1)

    return nc


_CACHE: dict[str, object] = {}


def _get_nc() -> bass.Bass:
    if "nc" not in _CACHE:
        nc = bass.Bass()
        build(nc, F_LIST)
        _CACHE["nc"] = nc
    return _CACHE["nc"]  # type: ignore[return-value]


def run_spmd(in_maps, trace=False, **kw):
    return run_bass_kernel_spmd(
        _get_nc(), in_maps, core_ids=list(range(N_CORES)), trace=trace, **kw
    )


def to_planar(shard: np.ndarray, f_list=F_LIST) -> np.ndarray:
    """[n_pad, 3] f32 -> flat [P*3*sum(f)] planar per-tile blocks."""
    parts = []
    pos = 0
    for f in f_list:
        blk = shard[pos : pos + P * f].reshape(P, f, 3).transpose(0, 2, 1)
        parts.append(blk.reshape(-1))
        pos += P * f
    return np.ascontiguousarray(np.concatenate(parts))


def from_planar(dev_out: np.ndarray, f_list=F_LIST) -> np.ndarray:
    """flat [P*16*sum(f)] (any dtype) -> [n_pad, 16] f32."""
    arr = np.asarray(dev_out, dtype=np.float32).reshape(-1)
    outs = []
    pos = 0
    for f in f_list:
        blk = arr[pos : pos + P * KD * f].reshape(P, KD, f).transpose(0, 2, 1)
        outs.append(blk.reshape(P * f, KD))
        pos += P * KD * f
    return np.concatenate(outs)


def make_in_maps(vectors: np.ndarray):
    vectors = np.ascontiguousarray(np.asarray(vectors, dtype=np.float32))
    assert vectors.shape == (N_TOTAL, 3)
    shards = vectors.reshape(N_CORES, N_CORE, 3)
    in_maps = []
    for i in range(N_CORES):
        buf = np.zeros((N_PAD, 3), dtype=np.float32)
        buf[:N_CORE] = shards[i]
        in_maps.append({"vectors": to_planar(buf)})
    return in_maps


def kernel(vectors: np.ndarray) -> np.ndarray:
    vec32 = np.ascontiguousarray(np.asarray(vectors, dtype=np.float32))
    res = run_spmd(make_in_maps(vec32))
    out = np.empty((N_TOTAL, K), dtype=np.float32)
    out[:, 0] = 1.0
    out[:, 1:4] = vec32  # degree-1 monomials are the input, exactly
    for i in range(N_CORES):
        out[i * N_CORE : (i + 1) * N_CORE, 4:] = from_planar(res.results[i]["out"])[
            :N_CORE
        ]
    return out
